# revision 90
# baseline (speedup 1.0000x reference)
"""NativeSparseAttention Trainium2 kernel (8-core SPMD).

Sharding: core c handles (b, kv) = (c // 4, c % 4); all three attention
branches, the gate/compress MLPs, and the k/v projections for that
(batch, kv-head) pair are fully independent across cores.

Numerics plan (v2):
  - k_cmp projection via 3-term bf16 split (wh*xh + wh*xl + wl*xh, fp32
    PSUM accumulate): ~4e-6 rms error, 1 cycle/row on the PE.
  - branch-1 selection chain: fp32 scores on the PE, ACT-table exp
    (~1.1e-5 relative) for p_grp; top-16 via max8/match_replace.
  - branches 1/2/3 outputs in bf16 (scores, probabilities, PV, staging);
    softmax normalization via an appended ones-column in V, applied
    together with the sigmoid gates at PSUM-eviction time through fused
    scalar_tensor_tensor ops.
  - sigmoid via tanh, so phase P only needs the gelu ACT table and the
    branch phase only the exp table (one table switch total).
"""

import sys, os

KPHASE = int(os.environ.get("KPHASE", "3"))  # 1=proj+compress, 2=+b1, 3=full

for _p in ("/opt/trn_rl_repo", "/root/.axon_site/_ro/trn_rl_repo"):
    if _p not in sys.path:
        sys.path.append(_p)

import numpy as np
import ml_dtypes

import concourse.bass as bass
import concourse.mybir as mybir
import concourse.tile as tile
from concourse import bacc
from concourse.bass_utils import run_bass_kernel_spmd

AF = mybir.ActivationFunctionType
ALU = mybir.AluOpType
F32 = mybir.dt.float32
BF16 = mybir.dt.bfloat16

# Model dims (hardcoded to the reference problem)
B, T, DM = 2, 1024, 2048
NQ, NKV, DH = 16, 4, 128
BLK, STRIDE, TOPN, WIN = 32, 16, 16, 512
NREP = NQ // NKV
NB = 63
NBP = 64                    # padded block count (col 63 is dead)
MO = DM // 128
TB = T // 128
TC = T // 512
SCALE = DH ** -0.5
STARTS = np.minimum(np.arange(NB) * STRIDE, T - 1)

NCORES = 8


def _emit(nc, tc, d, out_dram):
    def ap(name):
        return d[name].ap()

    from contextlib import ExitStack
    _stk = ExitStack()
    consts = _stk.enter_context(tc.tile_pool(name="consts", bufs=1))
    pers = _stk.enter_context(tc.tile_pool(name="pers", bufs=1))

    # ---------------- constants ----------------
    cosb_sb = consts.tile([64, T], BF16)
    sinb_sb = consts.tile([64, T], BF16)
    ident_sb = consts.tile([128, 128], F32)
    identb_sb = consts.tile([128, 128], BF16)
    cw_sb = consts.tile([128, 256], BF16)      # caus01 | win01
    caus01_sb = cw_sb[:, 0:128]
    win01_sb = cw_sb[:, 128:256]
    # additive-mask constants: tibh[m,t]=1 iff t in 16-token half-block m
    # (exactly one m per t); tibpen = 30*tibh; trineg[m,n] = -30 iff block n
    # has not begun for tokens in half m (m < n), or n is the pad column.
    tibh_sb = consts.tile([64, T], BF16)
    tibpen_sb = consts.tile([64, T], BF16)
    trineg_sb = consts.tile([64, NBP], BF16)
    ones_sb = consts.tile([1, 128], F32)
    nc.vector.memset(ones_sb[:], 1.0)
    bneg30_sb = consts.tile([128, 1], F32, tag="bneg30")
    nc.vector.memset(bneg30_sb[:], -30.0)
    brv_sb = consts.tile([1, 129], F32)
    bcol_sb = consts.tile([128, 3], F32, tag="bcol")   # b1k | b1v | ck2_b
    b1k_sb = bcol_sb[:, 0:1]
    b1v_sb = bcol_sb[:, 1:2]
    ck2b_sb = bcol_sb[:, 2:3]
    ck2_sb = consts.tile([128, 128], F32, tag="ck2")
    cv2_sb = consts.tile([128, 129], F32, tag="cv2")
    gbr_sb = consts.tile([1, 12], BF16, tag="gbr")
    onesb_sb = consts.tile([1, 128], BF16, tag="onesb")
    nc.vector.memset(onesb_sb[:], 1.0)

    # ---------------- persistent activations ----------------
    qb_sb = pers.tile([128, NREP, T], BF16, tag="qb")
    ql_sb = pers.tile([128, NREP, T], BF16, tag="ql")
    kslcT = pers.tile([128, T], BF16, tag="kslcT")
    kwinT = pers.tile([128, T], BF16, tag="kwinT")
    vslc = pers.tile([128, TB, 129], BF16, tag="vslc")
    vwin = pers.tile([128, TB, 129], BF16, tag="vwin")
    nc.vector.memset(vslc[:, :, 128:129], 1.0)
    nc.vector.memset(vwin[:, :, 128:129], 1.0)
    gates = pers.tile([128, TB, 12], F32, tag="gates")
    ksumT = pers.tile([128, NBP], F32, tag="ksumT")
    vsuma = pers.tile([NBP, 129], F32, tag="vsuma")
    ksum_bf = pers.tile([128, NBP], BF16, tag="ksumbf")
    ksum_lo = pers.tile([128, NBP], BF16, tag="ksumlo")
    vsuma_bf = pers.tile([NBP, 129], BF16, tag="vsumabf")
    pgrp = pers.tile([128, TB, NBP], F32, tag="pgrp")
    # whole-program work pools
    bev = _stk.enter_context(tc.tile_pool(name="bev", bufs=3))
    accp = _stk.enter_context(tc.tile_pool(name="accp", bufs=1))
    ps_aux = _stk.enter_context(tc.tile_pool(name="ps_aux", bufs=1, space="PSUM"))
    ps_sc = _stk.enter_context(tc.tile_pool(name="ps_sc", bufs=2, space="PSUM"))
    ps_pv = _stk.enter_context(tc.tile_pool(name="ps_pv", bufs=2, space="PSUM"))

    # ---------------- phase-P pools ----------------
    _stk_ck = ExitStack()
    ck1_pool = _stk_ck.enter_context(tc.tile_pool(name="ck1p", bufs=1))
    ck1_sb = ck1_pool.tile([128, BLK, 128], F32, tag="ck1")
    cv1_sb = ck1_pool.tile([128, BLK, 128], BF16, tag="cv1")
    _stk_P = ExitStack()
    projp = _stk_P.enter_context(tc.tile_pool(name="projp", bufs=1))
    wstrm = _stk_P.enter_context(tc.tile_pool(name="wstrm", bufs=1))
    pev = _stk_P.enter_context(tc.tile_pool(name="pev", bufs=2))
    ps_proj = _stk_P.enter_context(tc.tile_pool(name="ps_proj", bufs=2, space="PSUM"))
    _stk_ql = ExitStack()
    _stk_xl = ExitStack()
    xlp = _stk_xl.enter_context(tc.tile_pool(name="xlp", bufs=1))
    xlev = _stk_xl.enter_context(tc.tile_pool(name="xlev", bufs=2))
    cos_sb = xlp.tile([64, T], F32)
    sin_sb = xlp.tile([64, T], F32)
    wl_sb = wstrm.tile([128, MO, 128], BF16, tag="wcur", name="wl")
    wpc = {}
    for _wi in (2, 3, 4):
        wpc[_wi] = wstrm.tile([128, MO, 128], BF16, tag=f"w{_wi}",
                              name=f"w{_wi}")

    xh_sb = projp.tile([128, MO, T], BF16, tag="xh")
    wh_sb = projp.tile([128, MO, 128], BF16, tag="wh")
    gw_sb = projp.tile([128, MO, 12], BF16, tag="gw")
    wp5_sb = projp.tile([128, MO, 128], BF16, tag="wp5")
    wp6_sb = projp.tile([128, MO, 128], BF16, tag="wp6")
    kcmpT = projp.tile([128, T], F32, tag="kcmpT")
    vcmpT = projp.tile([128, T], BF16, tag="vcmpT")

    # --- critical-path DMAs (sync queue): k_win/v_win weights, x_hi, q ---
    nc.sync.dma_start(wp5_sb[:], ap("wTb")[5])
    nc.sync.dma_start(xh_sb[:, :, 0:256], ap("xTb")[0])
    nc.sync.dma_start(wp6_sb[:], ap("wTb")[6])
    for ch in range(1, 4):
        sl = slice(ch * 256, (ch + 1) * 256)
        nc.sync.dma_start(xh_sb[:, :, sl], ap("xTb")[ch])
    nc.sync.dma_start(wh_sb[:], ap("wTb")[0])
    nc.sync.dma_start(wl_sb[:], ap("wTb")[1])
    xl_chunks = []
    for ch in range(4):
        xlc = xlp.tile([128, MO, 256], BF16, tag=f"xl{ch % 2}", name="xlc")
        nc.sync.dma_start(xlc[:], ap("xTl")[ch])
        xl_chunks.append(xlc)
    # ordered by first use on the critical path to selection:
    # v_cmp proj -> compress -> gates -> pgrp -> slc projections
    nc.sync.dma_start(wpc[2][:], ap("wTb")[2])
    nc.sync.dma_start(ck1_sb[:], ap("ck1_wT"))
    nc.sync.dma_start(cv1_sb[:], ap("cv1_wTb"))
    nc.sync.dma_start(gw_sb[:], ap("gwTb"))
    nc.sync.dma_start(ident_sb[:], ap("ident"))
    nc.sync.dma_start(tibh_sb[:], ap("tibh"))
    nc.sync.dma_start(tibpen_sb[:], ap("tibpen"))
    nc.sync.dma_start(trineg_sb[:], ap("trineg"))
    nc.sync.dma_start(qb_sb[:], ap("qTb").rearrange("g p t -> p g t"))
    nc.sync.dma_start(ql_sb[:], ap("qTl").rearrange("g p t -> p g t"))
    nc.sync.dma_start(wpc[3][:], ap("wTb")[3])
    nc.sync.dma_start(wpc[4][:], ap("wTb")[4])

    # --- secondary DMAs (scalar queue): only rope tables up front; the rest
    # are issued after the first k_win piece so they queue behind it and do
    # not front-run the head-critical sync-queue transfers.
    nc.scalar.dma_start(cosb_sb[:], ap("cossinTb")[0:64])
    nc.scalar.dma_start(sinb_sb[:], ap("cossinTb")[64:128])

    def emit_secondary_dmas():
        nc.scalar.dma_start(cw_sb[:], ap("causwin"))
        nc.scalar.dma_start(cos_sb[:], ap("cossinT")[0:64])
        nc.scalar.dma_start(sin_sb[:], ap("cossinT")[64:128])
        nc.scalar.dma_start(ck2_sb[:], ap("ck2_wT"))
        nc.scalar.dma_start(bcol_sb[:], ap("bcol"))
        nc.scalar.dma_start(cv2_sb[:], ap("cv2_wTa"))
        nc.scalar.dma_start(brv_sb[:], ap("bias_row_v"))
        nc.scalar.dma_start(gbr_sb[:], ap("gb_row"))
        nc.scalar.dma_start(identb_sb[:], ap("identb"))

    def rope_evict_f32(ps, lo, w, out_T):
        sl = slice(lo, lo + w)
        c = cos_sb[:, sl]
        s = sin_sb[:, sl]
        ta = xlev.tile([64, 256], F32, tag="ropeA", name="ropeA")[:, 0:w]
        tb_ = xlev.tile([64, 256], F32, tag="ropeB", name="ropeB")[:, 0:w]
        nc.vector.tensor_tensor(ta[:], ps[0:64, :], c, op=ALU.mult)
        nc.vector.tensor_tensor(tb_[:], ps[64:128, :], s, op=ALU.mult)
        nc.vector.tensor_sub(out_T[0:64, sl], ta[:], tb_[:])
        nc.vector.tensor_tensor(ta[:], ps[0:64, :], s, op=ALU.mult)
        nc.vector.tensor_tensor(tb_[:], ps[64:128, :], c, op=ALU.mult)
        nc.vector.tensor_add(out_T[64:128, sl], ta[:], tb_[:])

    def rope_evict_bf16(ps, lo, w, out_T):
        # psum fp32 -> ACT bf16 copies -> DVE bf16 (fast-mode) rotate
        sl = slice(lo, lo + w)
        c = cosb_sb[:, sl]
        s = sinb_sb[:, sl]
        tlo = pev.tile([64, 512], BF16, tag="rblo", name="rblo")[:, 0:w]
        thi = pev.tile([64, 512], BF16, tag="rbhi", name="rbhi")[:, 0:w]
        nc.scalar.copy(tlo[:], ps[0:64, :])
        nc.vector.tensor_copy(thi[:], ps[64:128, :])
        ta = pev.tile([64, 512], BF16, tag="rbA", name="rbA")[:, 0:w]
        tb_ = pev.tile([64, 512], BF16, tag="rbB", name="rbB")[:, 0:w]
        nc.vector.tensor_tensor(ta[:], tlo[:], c, op=ALU.mult)
        nc.vector.tensor_tensor(tb_[:], thi[:], s, op=ALU.mult)
        nc.vector.tensor_sub(out_T[0:64, sl], ta[:], tb_[:])
        nc.vector.tensor_tensor(ta[:], tlo[:], s, op=ALU.mult)
        nc.vector.tensor_tensor(tb_[:], thi[:], c, op=ALU.mult)
        nc.vector.tensor_add(out_T[64:128, sl], ta[:], tb_[:])

    def emit_proj_bf16(w_wi, tck, kind):
        ps = ps_proj.tile([128, 512], F32, tag="P")
        for mo in range(MO):
            nc.tensor.matmul(ps[:], w_wi[:, mo, :],
                             xh_sb[:, mo, tck * 512:(tck + 1) * 512],
                             start=(mo == 0), stop=(mo == MO - 1))
        if kind == "kslc":
            rope_evict_bf16(ps, tck * 512, 512, kslcT)
        elif kind == "kwin":
            rope_evict_bf16(ps, tck * 512, 512, kwinT)
        elif kind == "vcmp":
            nc.scalar.copy(vcmpT[:, tck * 512:(tck + 1) * 512], ps[:])
        else:
            vdst = vslc if kind == "vslc" else vwin
            tmpb = pev.tile([128, 512], BF16, tag="vtmpb")
            nc.scalar.copy(tmpb[:], ps[:])
            pst = ps_aux.tile([128, 512], BF16, tag="Xb")
            for j in range(4):
                nc.tensor.transpose(pst[:, j * 128:(j + 1) * 128],
                                    tmpb[:, j * 128:(j + 1) * 128],
                                    identb_sb[:])
            nc.vector.tensor_copy(
                vdst[:, tck * 4:(tck + 1) * 4, 0:128],
                pst[:].rearrange("p (a b) -> p a b", b=128))

    # ---------- branch 3 (sliding window) ----------
    def emit_b3(g, o_win, hook=None):
        nhook = [0]
        psv3 = [None]

        def flush_b3(upto):
            if psv3[0] is not None:
                i0, n3, psv = psv3[0]
                nc.vector.tensor_copy(o_win[:, i0:i0 + n3, :], psv[:, 0:n3, :])
                psv3[0] = None

        for i in range(TB):
            if hook is not None and i in (1, 3, 5):
                hook(nhook[0]); nhook[0] += 1
            sl = slice(i * 128, (i + 1) * 128)
            kts = list(range(max(0, i - 4), i + 1))
            e3 = {}
            groups = [kts[j:j + 4] for j in range(0, len(kts), 4)]
            for grp in groups:
                ps = ps_b3.tile([128, 512], F32, tag="S3")
                for j, kt in enumerate(grp):
                    nc.tensor.matmul(ps[:, j * 128:(j + 1) * 128],
                                     kwinT[:, kt * 128:(kt + 1) * 128],
                                     qb_sb[:, g, sl], start=True, stop=True)
                et = e3pool.tile([128, 4, 128], BF16, tag=f"e3g_{grp[0] % 3}",
                                 name="e3g")
                nc.scalar.activation(
                    et[:, 0:len(grp), :],
                    ps[:, 0:len(grp) * 128].rearrange("p (a b) -> p a b", b=128),
                    AF.Exp)
                for j, kt in enumerate(grp):
                    if kt == i:
                        nc.gpsimd.tensor_tensor(et[:, j, :], et[:, j, :],
                                                caus01_sb[:], op=ALU.mult)
                    elif kt == i - 4:
                        nc.vector.tensor_tensor(et[:, j, :], et[:, j, :],
                                                win01_sb[:], op=ALU.mult)
                    e3[kt] = et[:, j, :]
            if psv3[0] is None or psv3[0][0] + psv3[0][1] != i or psv3[0][1] >= 3:
                flush_b3(i)
                psv3[0] = (i, 0, ps_aux.tile([128, 3, 129], F32, tag="X",
                                             name="psv3"))
            i0, n3, psv = psv3[0]
            for kt in kts:
                nc.tensor.matmul(psv[:, n3, :], e3[kt], vwin[:, kt, :],
                                 start=(kt == kts[0]), stop=(kt == kts[-1]))
            psv3[0] = (i0, n3 + 1, psv)
        flush_b3(TB)

    # k_win / v_win projections first (ordered so each piece's x chunks have
    # landed by the time the PE reaches it)
    for qch in range(2):
        sl256 = slice(qch * 256, (qch + 1) * 256)
        ps = ps_proj.tile([128, 512], F32, tag="P")
        for mo in range(MO):
            nc.tensor.matmul(ps[:, 0:256], wp5_sb[:, mo, :],
                             xh_sb[:, mo, sl256],
                             start=(mo == 0), stop=(mo == MO - 1))
        rope_evict_bf16(ps[:, 0:256], qch * 256, 256, kwinT)
        if qch == 0:
            emit_secondary_dmas()
    emit_proj_bf16(wp6_sb, 0, "vwin")
    emit_proj_bf16(wp5_sb, 1, "kwin")
    emit_proj_bf16(wp6_sb, 1, "vwin")

    # ---- k_cmp: 3-term bf16 split, fp32 accumulate, fp32 rope ----
    kc_state = {}

    def emit_kcmp_term(ch, t):
        sl = slice(ch * 256, (ch + 1) * 256)
        if t == 0:
            kc_state[ch] = ps_proj.tile([128, 512], F32, tag="P", name="Pk")
        ps = kc_state[ch]
        terms = ((wh_sb, xh_sb, sl), (wh_sb, xl_chunks[ch], slice(0, 256)),
                 (wl_sb, xh_sb, sl))
        w_t, x_t, xsl = terms[t]
        for mo in range(MO):
            nc.tensor.matmul(ps[:, 0:256], w_t[:, mo, :], x_t[:, mo, xsl],
                             start=(t == 0 and mo == 0),
                             stop=(t == 2 and mo == MO - 1))
        if t == 2:
            rope_evict_f32(ps[:, 0:256], ch * 256, 256, kcmpT)

    for ch in range(4):
        for t in range(3):
            emit_kcmp_term(ch, t)
    _stk_xl.close()

    # ---- v_cmp (bf16) ----
    emit_proj_bf16(wpc[2], 0, "vcmp")
    emit_proj_bf16(wpc[2], 1, "vcmp")

    # ---- compressed block summaries ----
    h_k = projp.tile([128, NBP], F32, tag="hk")
    h_v = projp.tile([128, NBP], F32, tag="hv")
    for w1t, srcT, bias1, h in ((ck1_sb, kcmpT, b1k_sb, h_k),
                                (cv1_sb, vcmpT, b1v_sb, h_v)):
        ps = ps_proj.tile([128, 512], F32, tag="P")
        for c in range(BLK):
            rhs = srcT[:, c:c + 16 * (NB - 1) + 1:16]
            nc.tensor.matmul(ps[:, 0:NB], w1t[:, c, :], rhs,
                             start=(c == 0), stop=(c == BLK - 1))
        nc.vector.memset(h[:, NB:NBP], 0.0)
        nc.scalar.activation(h[:, 0:NB], ps[:, 0:NB], AF.Gelu, bias=bias1[:])

    ps = ps_proj.tile([128, 512], F32, tag="P")
    nc.tensor.matmul(ps[:, 0:NBP], ck2_sb[:], h_k[:], start=True, stop=True)
    nc.scalar.activation(ksumT[:], ps[:, 0:NBP], AF.Identity, bias=ck2b_sb[:])

    psx = ps_aux.tile([128, 512], F32, tag="X")
    nc.tensor.matmul(psx[0:NBP, 0:129], h_v[:], cv2_sb[:], start=True, stop=False)
    nc.tensor.matmul(psx[0:NBP, 0:129], ones_sb[:, 0:NBP], brv_sb[:],
                     start=False, stop=True)
    nc.vector.tensor_copy(vsuma[:], psx[0:NBP, 0:129])
    nc.vector.tensor_copy(ksum_bf[:], ksumT[:])
    nc.vector.tensor_copy(vsuma_bf[:], vsuma[:])
    nc.vector.tensor_tensor(ksum_lo[:], ksumT[:], ksum_bf[:], op=ALU.subtract)

    # ---- gates: sigmoid(z) = 0.5*tanh(z/2) + 0.5 ----
    for tb in range(TB):
        psg = ps_aux.tile([128, 512], F32, tag="X")
        for mo in range(MO):
            nc.tensor.matmul(psg[:, 0:12],
                             xh_sb[:, mo, tb * 128:(tb + 1) * 128],
                             gw_sb[:, mo, :], start=(mo == 0), stop=False)
        nc.tensor.matmul(psg[:, 0:12], onesb_sb[:], gbr_sb[:],
                         start=False, stop=True)
        nc.scalar.activation(gates[:, tb, :], psg[:, 0:12], AF.Tanh, scale=0.5)
    nc.vector.tensor_scalar(gates[:], gates[:], 0.5, 0.5,
                            op0=ALU.mult, op1=ALU.add)

    if KPHASE < 2:
        _stk_ql.close()
        _stk_P.close()
        _stk_ck.close()
        _stk.close()
        return

    # ===== phase 2a: p_grp for all heads + slc projections + eA transposes =====
    pieces = [(3, 0, "kslc"), (3, 1, "kslc"), (4, 0, "vslc"), (4, 1, "vslc")]
    e_Ts = []
    for g in range(NREP):
        # high-precision scores, query-major, via 3-term bf16 split + penalty
        pss = ps_sc.tile([128, 512], F32, tag="S")
        for tb in range(TB):
            qh = qb_sb[:, g, tb * 128:(tb + 1) * 128]
            ql = ql_sb[:, g, tb * 128:(tb + 1) * 128]
            dst = pss[:, tb * 64:(tb + 1) * 64]
            nc.tensor.matmul(dst, qh, ksum_bf[:], start=True, stop=False)
            nc.tensor.matmul(dst, ql, ksum_bf[:], start=False, stop=False)
            nc.tensor.matmul(dst, qh, ksum_lo[:], start=False, stop=False)
            nc.tensor.matmul(dst, tibh_sb[:, tb * 128:(tb + 1) * 128],
                             trineg_sb[:], start=False, stop=True)
        eA = bev.tile([128, TB, NBP], F32, tag="eA")
        nc.scalar.activation(eA[:].rearrange("p a b -> p (a b)"), pss[:], AF.Exp)
        S = bev.tile([128, TB, 1], F32, tag="pS")
        nc.vector.reduce_sum(S[:], eA[:], axis=mybir.AxisListType.X)
        r = bev.tile([128, TB, 1], F32, tag="pr")
        nc.vector.reciprocal(r[:], S[:])
        for tb in range(TB):
            if g == 0:
                nc.vector.tensor_scalar(pgrp[:, tb, :], eA[:, tb, :],
                                        r[:, tb, :], None, op0=ALU.mult)
            else:
                nc.vector.scalar_tensor_tensor(pgrp[:, tb, :], eA[:, tb, :],
                                               r[:, tb, :], pgrp[:, tb, :],
                                               op0=ALU.mult, op1=ALU.add)
        # block-major copy of eA for the branch-1 PV later (replaces the
        # block-major score recompute + exp)
        e_Tg = accp.tile([NBP, T], BF16, tag=f"eT{g}", name="e_Tg")
        e_Ts.append(e_Tg)
        for tb0 in (0, 4):
            pst = ps_aux.tile([128, 512], F32, tag="X")
            for j in range(4):
                nc.tensor.transpose(pst[0:NBP, j * 128:(j + 1) * 128],
                                    eA[:, tb0 + j, :], ident_sb[:])
            if tb0 == 0:
                nc.vector.tensor_copy(e_Tg[:, 0:512], pst[0:NBP, :])
            else:
                nc.scalar.copy(e_Tg[:, 512:1024], pst[0:NBP, :])
        # slc projection piece (keeps the PE busy through this phase)
        wi, tck, kind = pieces[g]
        emit_proj_bf16(wpc[wi], tck, kind)
    _stk_ql.close()
    _stk_P.close()
    _stk_ck.close()

    selp = _stk.enter_context(tc.tile_pool(name="selp", bufs=1))
    selx = selp.tile([NBP, T], BF16, tag="selx")
    epool = _stk.enter_context(tc.tile_pool(name="epool", bufs=3))
    # bf16 staging for branch outputs (col 128 = softmax row-sum); all three
    # are produced and consumed within one g and rotate across heads.
    obuf = _stk.enter_context(tc.tile_pool(name="obuf", bufs=2))
    e3pool = _stk.enter_context(tc.tile_pool(name="e3pool", bufs=2))
    # separate PSUM banks for branch 3 so its score->exp->mask->PV chain
    # doesn't serialize against branch 2's PSUM rotation
    ps_b3 = _stk.enter_context(tc.tile_pool(name="ps_b3", bufs=2, space="PSUM"))

    # branch 3 for head 0 first: its PE/Act work overlaps the DVE-heavy
    # selection below (it needs nothing from selection)
    o_win0 = obuf.tile([128, TB, 129], BF16, tag="owin", name="o_win0")
    emit_b3(0, o_win0)

    # ===== phase 2b: top-16 selection =====
    # selx[m, t] = 1 iff half-block m is covered by some selected block for
    # query t (block m or m-1 selected). Each token lies in exactly one half,
    # so 30*tibh @ selx yields an exact {0,+30} coverage bonus which the
    # exp bias of -30 turns into the multiplicative {exp(-30), 1} mask.
    for tb in range(TB):
        mx = bev.tile([128, 8], F32, tag="mx8")
        sw = bev.tile([128, NBP], F32, tag="selw")
        nc.vector.max(mx[:], pgrp[:, tb, :])
        nc.vector.match_replace(sw[:], mx[:], pgrp[:, tb, :], 0.0)
        nc.vector.max(mx[:], sw[:])
        nc.vector.match_replace(sw[:], mx[:], sw[:], 0.0)
        nc.vector.tensor_sub(sw[:], pgrp[:, tb, :], sw[:])
        nc.scalar.activation(sw[:], sw[:], AF.Sign)
        selh = bev.tile([128, NBP], F32, tag="selh", name="selh")
        nc.vector.tensor_copy(selh[:, 0:1], sw[:, 0:1])
        nc.vector.tensor_tensor(selh[:, 1:NBP], sw[:, 1:NBP],
                                sw[:, 0:NBP - 1], op=ALU.max)
        pst = ps_aux.tile([128, 512], F32, tag="X")
        nc.tensor.transpose(pst[0:NBP, 0:128], selh[:], ident_sb[:])
        if tb % 2 == 0:
            nc.vector.tensor_copy(selx[:, tb * 128:(tb + 1) * 128],
                                  pst[0:NBP, 0:128])
        else:
            nc.scalar.copy(selx[:, tb * 128:(tb + 1) * 128],
                           pst[0:NBP, 0:128])

    if KPHASE < 3:
        _stk.close()
        return

    # ===== phase 2c: per-head branch 3 + branch-1 output + branch 2 =====
    for g in range(NREP):
        e2 = {}
        e_T = e_Ts[g]
        if g == 0:
            o_win = o_win0
        else:
            o_win = obuf.tile([128, TB, 129], BF16, tag="owin", name="o_win")
        o_slc = obuf.tile([128, TB, 129], BF16, tag="oslc", name="o_slc")
        o_cmp = obuf.tile([128, TB, 129], BF16, tag="ocmp", name="o_cmp")

        def b2_tile(kt, tck, g=g, e2=e2):
            # queries before this key tile never read their columns:
            # restrict work to the live range [lo0, 512)
            lo0 = max(0, kt - 4 * tck) * 128
            sl = slice(tck * 512 + lo0, (tck + 1) * 512)
            ps = ps_sc.tile([128, 512], F32, tag="S")
            nc.tensor.matmul(ps[:, lo0:512],
                             kslcT[:, kt * 128:(kt + 1) * 128],
                             qb_sb[:, g, sl], start=True, stop=False)
            nc.tensor.matmul(ps[:, lo0:512],
                             tibpen_sb[:, kt * 128:(kt + 1) * 128],
                             selx[:, sl], start=False, stop=True)
            et = epool.tile([128, 512], BF16, tag=f"e2_{kt}_{tck}",
                            name="e2t")
            nc.scalar.activation(et[:, lo0:512], ps[:, lo0:512],
                                 AF.Exp, bias=bneg30_sb[:])
            if kt >= 4 * tck:
                # diagonal query tile: kill s > t (incl. selection leak
                # past t from blocks overlapping the query position)
                nc.vector.tensor_tensor(et[:, lo0:lo0 + 128],
                                        et[:, lo0:lo0 + 128],
                                        caus01_sb, op=ALU.mult)
            e2[(kt, tck)] = et

        def b2_pv(ii, tck, o_slc=o_slc, e2=e2):
            i0 = 4 * tck + 2 * ii
            psv = ps_pv.tile([128, 3, 129], F32, tag="V")
            for j in range(2):
                i = i0 + j
                lo = (i - 4 * tck) * 128
                for kt in range(i + 1):
                    nc.tensor.matmul(psv[:, j, :],
                                     e2[(kt, tck)][:, lo:lo + 128],
                                     vslc[:, kt, :], start=(kt == 0),
                                     stop=(kt == i))
            nc.vector.tensor_copy(o_slc[:, i0:i0 + 2, :], psv[:, 0:2, :])

        # branch 3, with branch-2 tck=0 work feeding the PE/Act queues in
        # the gaps of b3's score->exp->mask->PV chain (g=0's b3 already ran
        # overlapped with selection)
        def hook(t):
            if t == 0:
                b2_tile(0, 0); b2_tile(1, 0)
            elif t == 1:
                b2_tile(2, 0); b2_tile(3, 0)
            else:
                b2_pv(0, 0); b2_pv(1, 0)
        if g == 0:
            for _t in range(3):
                hook(_t)
        else:
            emit_b3(g, o_win, hook=hook)

        for tb0 in range(0, TB, 3):
            nb3 = min(3, TB - tb0)
            psv = ps_pv.tile([128, 3, 129], F32, tag="V")
            for j in range(nb3):
                tb = tb0 + j
                nc.tensor.matmul(psv[:, j, :], e_T[:, tb * 128:(tb + 1) * 128],
                                 vsuma_bf[:], start=True, stop=True)
            nc.vector.tensor_copy(o_cmp[:, tb0:tb0 + nb3, :], psv[:, 0:nb3, :])

        # partial combine: o_cmp and o_win contributions (o_slc comes below)
        wj = bev.tile([128, TB, 3], F32, tag="wj", name="wj")
        for j, o_un in ((0, o_cmp[:, :, 128:129]),
                        (2, o_win[:, :, 128:129])):
            rr = bev.tile([128, TB, 1], F32, tag="rr", name="rr")
            nc.vector.reciprocal(rr[:], o_un)
            nc.vector.tensor_tensor(wj[:, :, j:j + 1], rr[:],
                                    gates[:, :, 3 * g + j:3 * g + j + 1],
                                    op=ALU.mult)
        acc = accp.tile([128, TB, 128], BF16, tag=f"acc{g}", name="acc")
        for tb in range(TB):
            nc.vector.tensor_scalar(acc[:, tb, :], o_cmp[:, tb, 0:128],
                                    wj[:, tb, 0:1], None, op0=ALU.mult)
            nc.vector.scalar_tensor_tensor(acc[:, tb, :], o_win[:, tb, 0:128],
                                           wj[:, tb, 2:3], acc[:, tb, :],
                                           op0=ALU.mult, op1=ALU.add)
        for kt in range(8):
            b2_tile(kt, 1)
        for ii in range(2):
            b2_pv(ii, 1)

        # ---------- add the remaining o_slc contribution and store ----------
        wj1 = bev.tile([128, TB, 1], F32, tag="wj1", name="wj1")
        rr = bev.tile([128, TB, 1], F32, tag="rr", name="rr")
        nc.vector.reciprocal(rr[:], o_slc[:, :, 128:129])
        nc.vector.tensor_tensor(wj1[:], rr[:],
                                gates[:, :, 3 * g + 1:3 * g + 2], op=ALU.mult)
        for tb in range(TB):
            nc.vector.scalar_tensor_tensor(acc[:, tb, :], o_slc[:, tb, 0:128],
                                           wj1[:, tb, 0:1], acc[:, tb, :],
                                           op0=ALU.mult, op1=ALU.add)
        nc.sync.dma_start(out_dram.ap()[g], acc[:])

    _stk.close()


def _build_program():
    nc = bacc.Bacc("TRN2", target_bir_lowering=False, debug=False,
                   num_devices=NCORES)
    dram = {}

    def din(name, shape, dtype=F32):
        dram[name] = nc.dram_tensor(name, list(shape), dtype, kind="ExternalInput")

    din("xTb", (4, 128, MO, 256), BF16)
    din("xTl", (4, 128, MO, 256), BF16)
    din("qTb", (NREP, 128, T), BF16)
    din("qTl", (NREP, 128, T), BF16)
    din("wTb", (7, 128, MO, 128), BF16)
    din("gwTb", (128, MO, 12), BF16)
    din("gb_row", (1, 12), BF16)
    din("cossinT", (128, T))
    din("cossinTb", (128, T), BF16)
    din("ck1_wT", (128, BLK, 128))
    din("cv1_wTb", (128, BLK, 128), BF16)
    din("bcol", (128, 3))
    din("ck2_wT", (128, 128))
    din("cv2_wTa", (128, 129))
    din("bias_row_v", (1, 129))
    din("tibh", (64, T), BF16)
    din("tibpen", (64, T), BF16)
    din("trineg", (64, NBP), BF16)
    din("causwin", (128, 256), BF16)
    din("ident", (128, 128))
    din("identb", (128, 128), BF16)
    out_dram = nc.dram_tensor("out", [NREP, 128, TB, DH], BF16,
                              kind="ExternalOutput")

    with tile.TileContext(nc) as tc:
        _emit(nc, tc, dram, out_dram)
    nc.compile()
    return nc


_PROGRAM = None


def _get_program():
    global _PROGRAM
    if _PROGRAM is None:
        _PROGRAM = _build_program()
    return _PROGRAM


def _host_inputs(inputs):
    x = np.asarray(inputs["x"], np.float32)
    q = np.asarray(inputs["q"], np.float32)
    gate_w = np.asarray(inputs["gate_w"], np.float32)
    gate_b = np.asarray(inputs["gate_b"], np.float32)
    block_pos = np.asarray(inputs["block_pos"], np.float32)

    half = DH // 2
    pos = np.arange(T, dtype=np.float32)
    inv = (1.0 / (10000.0 ** (np.arange(half, dtype=np.float32) / half))).astype(np.float32)
    ang = (pos[:, None] * inv[None, :]).astype(np.float32)
    cosT = np.cos(ang.astype(np.float64)).astype(np.float32).T.copy()
    sinT = np.sin(ang.astype(np.float64)).astype(np.float32).T.copy()

    t_idx = np.arange(T)
    # half-block membership (exactly one m per token) and begun-block penalty
    tibh_f = (t_idx[None, :] // 16 == np.arange(64)[:, None]).astype(np.float32)
    tibh = tibh_f.astype(ml_dtypes.bfloat16)
    tibpen = (30.0 * tibh_f).astype(ml_dtypes.bfloat16)
    m_i, n_i = np.arange(64)[:, None], np.arange(NBP)[None, :]
    trineg = (-30.0 * ((m_i < n_i) | (n_i == NBP - 1))).astype(ml_dtypes.bfloat16)
    loc = np.arange(128)
    caus01 = (loc[None, :] >= loc[:, None]).astype(np.float32)
    win01 = (loc[None, :] < loc[:, None]).astype(np.float32)
    causwin = np.concatenate([caus01, win01], 1).astype(ml_dtypes.bfloat16)
    ident = np.eye(128, dtype=np.float32)

    ws = [np.asarray(inputs[k], np.float32) for k in
          ("wk_cmp", "wv_cmp", "wk_slc", "wv_slc", "wk_win", "wv_win")]
    ck1_w = np.asarray(inputs["ck1_w"], np.float32)
    cv1_w = np.asarray(inputs["cv1_w"], np.float32)
    bp_flat = block_pos.reshape(-1)
    b1k = (np.asarray(inputs["ck1_b"], np.float32) + ck1_w @ bp_flat).reshape(128, 1)
    b1v = (np.asarray(inputs["cv1_b"], np.float32) + cv1_w @ bp_flat).reshape(128, 1)
    ck1_wT = ck1_w.reshape(128, BLK, 128).transpose(2, 1, 0).copy()
    cv1_wT = cv1_w.reshape(128, BLK, 128).transpose(2, 1, 0).copy()
    ck2_wT = np.asarray(inputs["ck2_w"], np.float32).T.copy()
    ck2_b = np.asarray(inputs["ck2_b"], np.float32).reshape(128, 1)
    cv2_wTa = np.concatenate([np.asarray(inputs["cv2_w"], np.float32).T,
                              np.zeros((128, 1), np.float32)], 1)
    bias_row_v = np.concatenate([np.asarray(inputs["cv2_b"], np.float32),
                                 [1.0]]).astype(np.float32).reshape(1, 129)

    xs = x.astype(ml_dtypes.bfloat16)
    xl = (x - xs.astype(np.float32)).astype(ml_dtypes.bfloat16)

    cossinT = np.concatenate([cosT, sinT], 0)
    cossinTb = cossinT.astype(ml_dtypes.bfloat16)
    bcol = np.concatenate([b1k, b1v,
                           np.asarray(inputs["ck2_b"], np.float32).reshape(128, 1)],
                          1)

    in_maps = []
    for core in range(NCORES):
        b, kv = divmod(core, NKV)
        heads = [g * NKV + kv for g in range(NREP)]
        # (chunk, p, mo, 256): contiguous per-partition DMA runs
        xTb = np.ascontiguousarray(xs[b].reshape(4, 256, MO, 128)
                                   .transpose(0, 3, 2, 1))
        xTl = np.ascontiguousarray(xl[b].reshape(4, 256, MO, 128)
                                   .transpose(0, 3, 2, 1))
        qh = q[b, heads] * SCALE
        qT = np.ascontiguousarray(qh.transpose(0, 2, 1))
        qTb = qT.astype(ml_dtypes.bfloat16)
        qTl = (qT - qTb.astype(np.float32)).astype(ml_dtypes.bfloat16)
        wk = ws[0][kv * DH:(kv + 1) * DH]
        wk_hi = wk.astype(ml_dtypes.bfloat16)
        wk_lo = (wk - wk_hi.astype(np.float32)).astype(ml_dtypes.bfloat16)
        wTl = [wk_hi.astype(np.float32).T.reshape(MO, 128, DH),
               wk_lo.astype(np.float32).T.reshape(MO, 128, DH)]
        for w in ws[1:]:
            wTl.append(w[kv * DH:(kv + 1) * DH].T.reshape(MO, 128, DH))
        # (7, p, mo, d): contiguous per-partition DMA runs
        wTb = np.ascontiguousarray(
            np.stack(wTl).transpose(0, 2, 1, 3)).astype(ml_dtypes.bfloat16)
        cols = [h * 3 + j for h in heads for j in range(3)]
        gwTb = np.ascontiguousarray(
            gate_w[cols].T.reshape(MO, 128, 12).transpose(1, 0, 2)
        ).astype(ml_dtypes.bfloat16)
        gb_row = gate_b[cols].reshape(1, 12).astype(ml_dtypes.bfloat16)
        in_maps.append({
            "xTb": xTb, "xTl": xTl,
            "qTb": qTb, "qTl": qTl,
            "wTb": wTb, "gwTb": gwTb, "gb_row": gb_row,
            "cossinT": cossinT, "cossinTb": cossinTb,
            "ck1_wT": ck1_wT, "cv1_wTb": cv1_wT.astype(ml_dtypes.bfloat16),
            "bcol": bcol,
            "ck2_wT": ck2_wT, "cv2_wTa": cv2_wTa,
            "bias_row_v": bias_row_v,
            "tibh": tibh, "tibpen": tibpen, "trineg": trineg,
            "causwin": causwin, "ident": ident,
            "identb": ident.astype(ml_dtypes.bfloat16),
        })
    return in_maps


def kernel(**inputs) -> np.ndarray:
    nc = _get_program()
    in_maps = _host_inputs(inputs)
    res = run_bass_kernel_spmd(nc, in_maps, list(range(NCORES)))
    out = np.empty((B, NQ, T, DH), np.float32)
    for core in range(NCORES):
        b, kv = divmod(core, NKV)
        oc = res.results[core]["out"]
        for g in range(NREP):
            out[b, g * NKV + kv] = (np.asarray(oc[g]).astype(np.float32)
                                    .transpose(1, 0, 2).reshape(T, DH))
    return out


if __name__ == "__main__":
    _get_program()
    print("program built + compiled OK")

